# revision 13
# baseline (speedup 1.0000x reference)
"""Trainium2 Bass kernel for nn_MultiHeadSelfAttention_88725434400988.

R2 bisect: baseline projection/v-proj/startup + NEW attention section
(pipelined emission, shared dpr/av bank, 128-part resid, fp16 out).
"""
import numpy as np

B, S, F, E, A, NH = 256, 8, 32, 64, 64, 2
NCORES = 8
BC = B // NCORES            # 32 batches per core
ROWS = BC * S               # 256 projection rows
CD = F * E                  # 2048 contraction dim
ND = A * F * NH             # 4096 projection cols
KTILES = CD // 128          # 16
TTILES = ND // 128          # 32 column tiles per weight
NB = BC * NH                # 64 attention batches per core
WCHUNK = 2                  # weight tiles per DMA
GT = 2                      # projection tiles batched per psum/copy group

_NC_CACHE = None


def build_bass():
    import concourse.bacc as bacc
    import concourse.tile as tile
    from concourse import mybir

    f16 = mybir.dt.float16
    bf16 = mybir.dt.bfloat16
    f32 = mybir.dt.float32
    Exp = mybir.ActivationFunctionType.Exp
    Relu = mybir.ActivationFunctionType.Relu

    nc = bacc.Bacc("TRN2", target_bir_lowering=False, debug=False)

    # host-prepped layouts (see make_in_maps)
    hst_d = nc.dram_tensor("hst", [128, KTILES, ROWS], f16, kind="ExternalInput")
    hsv_d = nc.dram_tensor("hsv", [128, NB // 2, 128], f16, kind="ExternalInput")
    wq_d = nc.dram_tensor("wq", [128, TTILES, KTILES * 128], f16,
                          kind="ExternalInput")
    wk_d = nc.dram_tensor("wk", [128, TTILES, KTILES * 128], f16,
                          kind="ExternalInput")
    wv_d = nc.dram_tensor("wv", [E, 2 * A], f16, kind="ExternalInput")
    wres_d = nc.dram_tensor("wres", [2 * A, E], f16, kind="ExternalInput")
    bias_d = nc.dram_tensor("bias", [E, 1], f32, kind="ExternalInput")
    out_d = nc.dram_tensor("out", [128, BC // 4, 512], f16,
                           kind="ExternalOutput")

    with tile.TileContext(nc) as tc:
        from contextlib import ExitStack
        with ExitStack() as ctx:
            singles = ctx.enter_context(tc.tile_pool(name="singles", bufs=1))

            # ---- constants / persistent tiles ----
            ones_bf = singles.tile([128, A], bf16)
            nc.vector.memset(ones_bf, 1.0)

            hsT = singles.tile([128, KTILES, ROWS], f16)
            for c in range(4):
                nc.gpsimd.dma_start(hsT[:, c * 4:(c + 1) * 4, :],
                                    hst_d[:, c * 4:(c + 1) * 4, :])
            hsv = singles.tile([128, NB // 2, 128], f16)
            nc.gpsimd.dma_start(hsv[:, :, :], hsv_d[:])

            wv_sb = singles.tile([128, 2 * A], f16)
            nc.gpsimd.dma_start(wv_sb[0:64, :], wv_d[:])
            nc.gpsimd.dma_start(wv_sb[64:128, :], wv_d[:])

            wres_sb = singles.tile([128, 2, E], f16)
            for half in range(2):
                for jh in range(2):
                    nc.gpsimd.dma_start(
                        wres_sb[half * 64:(half + 1) * 64, jh, :],
                        wres_d[jh * 64:(jh + 1) * 64, :])

            bias_sb = singles.tile([128, 1], f32)
            nc.gpsimd.dma_start(bias_sb[0:64, :], bias_d[:])
            nc.gpsimd.dma_start(bias_sb[64:128, :], bias_d[:])

            qt = singles.tile([64, 2, BC, NH, 128], f16)
            kt_ = singles.tile([64, 2, BC, NH, 128], f16)
            v_all = singles.tile([128, NB, 2, A], bf16)
            ut = singles.tile([128, BC, 2, 128], f16)  # (nh,a) x (b, jh, f*4+sp)

            # ---- Q/K projection + batched gathers ----
            with tc.tile_pool(name="wtile", bufs=4) as w_pool, \
                 tc.tile_pool(name="stage", bufs=2) as st_pool, \
                 tc.tile_pool(name="pp", bufs=3, space="PSUM") as pp_pool:
                for widx, (w_d, dest) in enumerate(((wq_d, qt), (wk_d, kt_))):
                    stage = st_pool.tile([128, BC, NH, 128], f16,
                                         name="stage", tag="stage")
                    for tg in range(TTILES // WCHUNK):
                        dma_eng = nc.sync if tg % 2 == 0 else nc.gpsimd
                        wt = w_pool.tile([128, WCHUNK, KTILES, 128], f16,
                                         name="wt", tag="wt")
                        wsrc = (w_d[:, tg * WCHUNK:(tg + 1) * WCHUNK, :]
                                .rearrange("p t (kt c) -> p t kt c", c=128))
                        if widx == 0 and tg == 0:
                            # split the gating first chunk so MMs start
                            # after half the data
                            for h2 in range(WCHUNK):
                                dma_eng.dma_start(
                                    wt[:, h2:h2 + 1, :, :],
                                    wsrc[:, h2:h2 + 1, :, :])
                        else:
                            dma_eng.dma_start(wt[:, :, :, :], wsrc)
                        for gi in range(WCHUNK // GT):
                            pp = pp_pool.tile([128, GT, ROWS], f32)
                            for ti in range(GT):
                                for kt in range(KTILES):
                                    nc.tensor.matmul(
                                        pp[:, ti, :],
                                        lhsT=wt[:, gi * GT + ti, kt, :],
                                        rhs=hsT[:, kt, :],
                                        start=(kt == 0),
                                        stop=(kt == KTILES - 1))
                            # psum free (ti, b, nh, sp) -> iterate (bn, ti, sp)
                            src = pp.rearrange(
                                "p ti (b n sp) -> p (b n) ti sp", n=NH, sp=4)
                            t0 = tg * WCHUNK + gi * GT
                            dv = dest[:, 0, :, :, :].rearrange(
                                "p b n (f sp) -> p (b n) f sp", sp=4)
                            sv = stage[:, :, :, :].rearrange(
                                "p b n (f sp) -> p (b n) f sp", sp=4)
                            nc.vector.tensor_copy(
                                dv[:, :, t0:t0 + GT, :], src[0:64])
                            nc.vector.tensor_copy(
                                sv[64:128, :, t0:t0 + GT, :], src[64:128])
                    # partition shift 64..127 -> 0..63 via SBUF->SBUF DMA,
                    # chunked; the Wq shift overlaps the Wk stream.
                    nchunks = 2 if widx == 0 else 4
                    bchunk = BC // nchunks
                    for c in range(nchunks):
                        eng = nc.gpsimd if widx == 0 else nc.sync
                        eng.dma_start(
                            dest[:, 1, c * bchunk:(c + 1) * bchunk, :, :],
                            stage[64:128, c * bchunk:(c + 1) * bchunk, :, :])

            # ---- v projection. Per 4-bn group: the two PE row-group
            # streams go to separate psum tiles (as in the per-pair version
            # that is known to run), two bpairs batched per tile, and the
            # two evacuation casts split across Vector/Scalar so the PE
            # stays dense across the proj->attention transition. ----
            with tc.tile_pool(name="vps", bufs=2, space="PSUM") as vps_pool:
                for q in range(NB // 4):
                    vp = [vps_pool.tile([128, 2, 2, 64], f32, name=f"vp{i}",
                                        tag=f"vp{i}") for i in range(2)]
                    for j in range(4):
                        pi = j % 2
                        nc.tensor.matmul(
                            vp[pi][:, j // 2, :, :],
                            lhsT=hsv[pi * 64:(pi + 1) * 64, 2 * q + j // 2, :],
                            rhs=wv_sb[pi * 64:(pi + 1) * 64, :],
                            start=True, stop=True,
                            tile_position=(pi * 64, 0))
                    for pi in range(2):
                        cp = (nc.vector.tensor_copy if pi == 0
                              else nc.scalar.copy)
                        cp(v_all[:, 4 * q + pi:4 * q + 4:2, :, :],
                           vp[pi][:, :, :, :])

            # ---- PE warm-up burst ----
            with tc.tile_pool(name="warm", bufs=1, space="PSUM") as wm_pool:
                wm = wm_pool.tile([A, ROWS], f32)
                for wi in range(12):
                    nc.tensor.matmul(
                        wm[:, :], lhsT=ones_bf[:, :], rhs=hsT[:, 0, :],
                        start=(wi == 0), stop=(wi == 11))

            # ---- attention (transpose-free, Z^T layout), software
            # pipelined: z/exp for b+1 are emitted before denom/AV of b ----
            with tc.tile_pool(name="zps", bufs=2, space="PSUM") as z_pool, \
                 tc.tile_pool(name="dav", bufs=2, space="PSUM") as da_pool, \
                 tc.tile_pool(name="rps", bufs=2, space="PSUM") as r_pool, \
                 tc.tile_pool(name="expz", bufs=3) as e_pool, \
                 tc.tile_pool(name="reps", bufs=3) as rp_pool, \
                 tc.tile_pool(name="fo", bufs=2) as f_pool:

                live = {}

                def emit_z(b):
                    zt = z_pool.tile([128, NH, 2, 256], f32, name="zt",
                                     tag="zt")
                    for nh in range(NH):
                        for h in range(2):
                            nc.tensor.matmul(
                                zt[:, nh, h, :],
                                lhsT=kt_[:, h, b, nh, :],
                                rhs=qt[:, :, b, nh, :],
                                start=True, stop=True)
                    ez = e_pool.tile([128, NH, 2, 256], bf16, name="ez",
                                     tag="ez")
                    nc.scalar.activation(
                        ez.rearrange("p a b c -> p (a b c)"),
                        zt.rearrange("p a b c -> p (a b c)"), Exp)
                    live[b] = ez

                def emit_tail(b):
                    ez = live.pop(b)
                    da = da_pool.tile([128, 2, 256], f32, name="da", tag="da")
                    for nh in range(NH):
                        for h in range(2):
                            nc.tensor.matmul(
                                da[nh * 64:(nh + 1) * 64, 0, :],
                                lhsT=ones_bf[:, :],
                                rhs=ez[:, nh, h, :],
                                start=(h == 0), stop=(h == 1),
                                tile_position=(0, nh * 64))
                    for nh in range(NH):
                        bn = b * NH + nh
                        for kk in range(2):
                            nc.tensor.matmul(
                                da[nh * 64:(nh + 1) * 64, 1, :],
                                lhsT=v_all[:, bn, kk, :],
                                rhs=ez[:, nh, kk, :],
                                start=(kk == 0), stop=(kk == 1),
                                tile_position=(0, nh * 64))
                    rep = rp_pool.tile([128, 256], f32, name="rep", tag="rep")
                    nc.vector.reciprocal_approx_fast(rep[:, :], da[:, 0, :])
                    nc.vector.tensor_mul(
                        ut[:, b, :, :].rearrange("p a b -> p (a b)"),
                        da[:, 1, :], rep[:, :])
                    if b % 4 == 3:
                        bg = b // 4
                        rp = r_pool.tile([128, 512], f32, name="rp", tag="rp")
                        for nh in range(NH):
                            for jh in range(2):
                                nc.tensor.matmul(
                                    rp[nh * 64:(nh + 1) * 64, :],
                                    lhsT=wres_sb[nh * 64:(nh + 1) * 64, jh, :],
                                    rhs=ut[nh * 64:(nh + 1) * 64,
                                           bg * 4:(bg + 1) * 4, jh, :],
                                    start=(jh == 0), stop=(jh == 1),
                                    tile_position=(nh * 64, nh * 64))
                        fo = f_pool.tile([128, 512], f16, name="fo", tag="fo")
                        nc.scalar.activation(fo[:, :], rp[:, :], Relu,
                                             bias=bias_sb[:, :])
                        nc.sync.dma_start(out_d[:, bg, :], fo[:, :])

                emit_z(0)
                for b in range(BC):
                    if b + 1 < BC:
                        emit_z(b + 1)
                    emit_tail(b)
    nc.compile()
    return nc


def _get_nc():
    global _NC_CACHE
    if _NC_CACHE is None:
        _NC_CACHE = build_bass()
    return _NC_CACHE


def _prep_weight(W):
    # (CD, ND) -> (128, TTILES, KTILES*128): [p, t, kt*128+j] = W[kt*128+p, t*128+j]
    return np.ascontiguousarray(
        W.astype(np.float16).reshape(KTILES, 128, TTILES, 128)
        .transpose(1, 2, 0, 3).reshape(128, TTILES, KTILES * 128))


def make_in_maps(Hs, Wq, Wk, Wv, Wres_w, Wres_b):
    wq16 = _prep_weight(Wq)
    wk16 = _prep_weight(Wk)
    wv16 = Wv.astype(np.float16)
    wres16 = Wres_w.astype(np.float16)
    bias = Wres_b.astype(np.float32).reshape(E, 1)
    hs16 = Hs.astype(np.float16)
    maps = []
    for c in range(NCORES):
        sh = hs16[c * BC:(c + 1) * BC]                      # (BC, S, CD)
        hs2d = sh.reshape(ROWS, CD)
        hst = np.ascontiguousarray(
            hs2d.reshape(ROWS, KTILES, 128).transpose(2, 1, 0))
        # v rows in sigma' order (f*4+sp):
        # hsv[pi*64+e, q, f*4+sp] = Hs[b, nh*4+sp, f, e]; bn = 2q+pi = b*NH+nh
        arr = sh.reshape(NB, 4, F, E).transpose(0, 2, 1, 3).reshape(NB, 128, E)
        hsv = np.ascontiguousarray(
            arr.reshape(NB // 2, 2, 128, E).transpose(1, 3, 0, 2)
            .reshape(128, NB // 2, 128))
        maps.append({
            "hst": hst, "hsv": hsv,
            "wq": wq16, "wk": wk16, "wv": wv16, "wres": wres16, "bias": bias,
        })
    return maps


def _unpack_out(o):
    # o: (128, BC//4, 512) = ((nh, e), bg, (b4, f, sp)) -> (BC, S, F*E)
    o = np.asarray(o, dtype=np.float32).reshape(NH, E, BC // 4, 4, F, 4)
    return np.ascontiguousarray(
        o.transpose(2, 3, 0, 5, 4, 1)).reshape(BC, S, F * E)


def kernel(Hs, Wq, Wk, Wv, Wres_w, Wres_b):
    from concourse.bass_utils import run_bass_kernel_spmd
    nc = _get_nc()
    in_maps = make_in_maps(Hs, Wq, Wk, Wv, Wres_w, Wres_b)
    res = run_bass_kernel_spmd(nc, in_maps, list(range(NCORES)))
    out = np.concatenate(
        [_unpack_out(np.asarray(res.results[c]["out"]))
         for c in range(NCORES)], axis=0)
    return out.astype(np.float32)


if __name__ == "__main__":
    nc = build_bass()
    print("built OK; instructions:",
          sum(len(bb.instructions) for fn in nc.m.functions
              for bb in fn.blocks))


# revision 14
# speedup vs baseline: 1.0984x; 1.0984x over previous
"""Trainium2 Bass kernel for nn_MultiHeadSelfAttention_88725434400988.

R2 bisect: baseline projection/v-proj/startup + NEW attention section
(pipelined emission, shared dpr/av bank, 128-part resid, fp16 out).
"""
import numpy as np

B, S, F, E, A, NH = 256, 8, 32, 64, 64, 2
NCORES = 8
BC = B // NCORES            # 32 batches per core
ROWS = BC * S               # 256 projection rows
CD = F * E                  # 2048 contraction dim
ND = A * F * NH             # 4096 projection cols
KTILES = CD // 128          # 16
TTILES = ND // 128          # 32 column tiles per weight
NB = BC * NH                # 64 attention batches per core
WCHUNK = 2                  # weight tiles per DMA
GT = 2                      # projection tiles batched per psum/copy group

_NC_CACHE = None


def build_bass():
    import concourse.bacc as bacc
    import concourse.tile as tile
    from concourse import mybir

    f16 = mybir.dt.float16
    bf16 = mybir.dt.bfloat16
    f32 = mybir.dt.float32
    Exp = mybir.ActivationFunctionType.Exp
    Relu = mybir.ActivationFunctionType.Relu

    nc = bacc.Bacc("TRN2", target_bir_lowering=False, debug=False)

    # host-prepped layouts (see make_in_maps)
    hst_d = nc.dram_tensor("hst", [128, KTILES, ROWS], f16, kind="ExternalInput")
    hsv_d = nc.dram_tensor("hsv", [128, NB // 2, 128], f16, kind="ExternalInput")
    wq_d = nc.dram_tensor("wq", [128, TTILES, KTILES * 128], f16,
                          kind="ExternalInput")
    wk_d = nc.dram_tensor("wk", [128, TTILES, KTILES * 128], f16,
                          kind="ExternalInput")
    wv_d = nc.dram_tensor("wv", [E, 2 * A], f16, kind="ExternalInput")
    wres_d = nc.dram_tensor("wres", [2 * A, E], f16, kind="ExternalInput")
    bias_d = nc.dram_tensor("bias", [E, 1], f32, kind="ExternalInput")
    out_d = nc.dram_tensor("out", [128, BC // 4, 512], f16,
                           kind="ExternalOutput")

    with tile.TileContext(nc) as tc:
        from contextlib import ExitStack
        with ExitStack() as ctx:
            singles = ctx.enter_context(tc.tile_pool(name="singles", bufs=1))

            # ---- constants / persistent tiles ----
            ones_bf = singles.tile([128, A], bf16)
            nc.vector.memset(ones_bf, 1.0)

            hsT = singles.tile([128, KTILES, ROWS], f16)
            for c in range(4):
                nc.gpsimd.dma_start(hsT[:, c * 4:(c + 1) * 4, :],
                                    hst_d[:, c * 4:(c + 1) * 4, :])
            hsv = singles.tile([128, NB // 2, 128], f16)
            nc.gpsimd.dma_start(hsv[:, :, :], hsv_d[:])

            wv_sb = singles.tile([128, 2 * A], f16)
            nc.gpsimd.dma_start(wv_sb[0:64, :], wv_d[:])
            nc.gpsimd.dma_start(wv_sb[64:128, :], wv_d[:])

            wres_sb = singles.tile([128, 2, E], f16)
            for half in range(2):
                for jh in range(2):
                    nc.gpsimd.dma_start(
                        wres_sb[half * 64:(half + 1) * 64, jh, :],
                        wres_d[jh * 64:(jh + 1) * 64, :])

            bias_sb = singles.tile([128, 1], f32)
            nc.gpsimd.dma_start(bias_sb[0:64, :], bias_d[:])
            nc.gpsimd.dma_start(bias_sb[64:128, :], bias_d[:])

            qt = singles.tile([64, 2, BC, NH, 128], f16)
            kt_ = singles.tile([64, 2, BC, NH, 128], f16)
            v_all = singles.tile([128, NB, 2, A], bf16)
            ut = singles.tile([128, BC, 2, 128], f16)  # (nh,a) x (b, jh, f*4+sp)

            # ---- Q/K projection + batched gathers; the v projection
            # block sits between the two weight streams so its psum-evac
            # casts drain under the Wk stream and its matmuls bridge the
            # Wq->Wk DMA boundary. ----
            with tc.tile_pool(name="wtile", bufs=4) as w_pool, \
                 tc.tile_pool(name="stage", bufs=2) as st_pool, \
                 tc.tile_pool(name="pp", bufs=3, space="PSUM") as pp_pool, \
                 tc.tile_pool(name="vps", bufs=2, space="PSUM") as vps_pool:

                def v_proj_block():
                    # Per 4-bn group: the two PE row-group streams go to
                    # separate psum tiles (alternating row-group matmuls
                    # into ONE tile abort the run), two bpairs per tile,
                    # casts split across Vector/Scalar to keep PE dense.
                    for q in range(NB // 4):
                        vp = [vps_pool.tile([128, 2, 2, 64], f32,
                                            name=f"vp{i}", tag=f"vp{i}")
                              for i in range(2)]
                        for j in range(4):
                            pi = j % 2
                            nc.tensor.matmul(
                                vp[pi][:, j // 2, :, :],
                                lhsT=hsv[pi * 64:(pi + 1) * 64,
                                         2 * q + j // 2, :],
                                rhs=wv_sb[pi * 64:(pi + 1) * 64, :],
                                start=True, stop=True,
                                tile_position=(pi * 64, 0))
                        for pi in range(2):
                            cp = (nc.vector.tensor_copy if pi == 0
                                  else nc.scalar.copy)
                            cp(v_all[:, 4 * q + pi:4 * q + 4:2, :, :],
                               vp[pi][:, :, :, :])

                for widx, (w_d, dest) in enumerate(((wq_d, qt), (wk_d, kt_))):
                    stage = st_pool.tile([128, BC, NH, 128], f16,
                                         name="stage", tag="stage")
                    for tg in range(TTILES // WCHUNK):
                        dma_eng = nc.sync if tg % 2 == 0 else nc.gpsimd
                        wt = w_pool.tile([128, WCHUNK, KTILES, 128], f16,
                                         name="wt", tag="wt")
                        wsrc = (w_d[:, tg * WCHUNK:(tg + 1) * WCHUNK, :]
                                .rearrange("p t (kt c) -> p t kt c", c=128))
                        if widx == 0 and tg == 0:
                            # split the gating first chunk so MMs start
                            # after half the data
                            for h2 in range(WCHUNK):
                                dma_eng.dma_start(
                                    wt[:, h2:h2 + 1, :, :],
                                    wsrc[:, h2:h2 + 1, :, :])
                        else:
                            dma_eng.dma_start(wt[:, :, :, :], wsrc)
                        for gi in range(WCHUNK // GT):
                            pp = pp_pool.tile([128, GT, ROWS], f32)
                            for ti in range(GT):
                                for kt in range(KTILES):
                                    nc.tensor.matmul(
                                        pp[:, ti, :],
                                        lhsT=wt[:, gi * GT + ti, kt, :],
                                        rhs=hsT[:, kt, :],
                                        start=(kt == 0),
                                        stop=(kt == KTILES - 1))
                            # psum free (ti, b, nh, sp) -> iterate (bn, ti, sp)
                            src = pp.rearrange(
                                "p ti (b n sp) -> p (b n) ti sp", n=NH, sp=4)
                            t0 = tg * WCHUNK + gi * GT
                            dv = dest[:, 0, :, :, :].rearrange(
                                "p b n (f sp) -> p (b n) f sp", sp=4)
                            sv = stage[:, :, :, :].rearrange(
                                "p b n (f sp) -> p (b n) f sp", sp=4)
                            nc.vector.tensor_copy(
                                dv[:, :, t0:t0 + GT, :], src[0:64])
                            nc.vector.tensor_copy(
                                sv[64:128, :, t0:t0 + GT, :], src[64:128])
                    # partition shift 64..127 -> 0..63 via SBUF->SBUF DMA,
                    # chunked; the Wq shift overlaps the Wk stream.
                    nchunks = 2 if widx == 0 else 4
                    bchunk = BC // nchunks
                    for c in range(nchunks):
                        eng = nc.gpsimd if widx == 0 else nc.sync
                        eng.dma_start(
                            dest[:, 1, c * bchunk:(c + 1) * bchunk, :, :],
                            stage[64:128, c * bchunk:(c + 1) * bchunk, :, :])
                    if widx == 0:
                        v_proj_block()

            # ---- PE warm-up burst ----
            with tc.tile_pool(name="warm", bufs=1, space="PSUM") as wm_pool:
                wm = wm_pool.tile([A, ROWS], f32)
                for wi in range(16):
                    nc.tensor.matmul(
                        wm[:, :], lhsT=ones_bf[:, :], rhs=hsT[:, 0, :],
                        start=(wi == 0), stop=(wi == 15))

            # ---- attention (transpose-free, Z^T layout), software
            # pipelined: z/exp for b+1 are emitted before denom/AV of b ----
            with tc.tile_pool(name="zps", bufs=2, space="PSUM") as z_pool, \
                 tc.tile_pool(name="dav", bufs=2, space="PSUM") as da_pool, \
                 tc.tile_pool(name="rps", bufs=2, space="PSUM") as r_pool, \
                 tc.tile_pool(name="expz", bufs=3) as e_pool, \
                 tc.tile_pool(name="reps", bufs=3) as rp_pool, \
                 tc.tile_pool(name="fo", bufs=2) as f_pool:

                live = {}

                def emit_z(b):
                    zt = z_pool.tile([128, NH, 2, 256], f32, name="zt",
                                     tag="zt")
                    for nh in range(NH):
                        for h in range(2):
                            nc.tensor.matmul(
                                zt[:, nh, h, :],
                                lhsT=kt_[:, h, b, nh, :],
                                rhs=qt[:, :, b, nh, :],
                                start=True, stop=True)
                    ez = e_pool.tile([128, NH, 2, 256], bf16, name="ez",
                                     tag="ez")
                    nc.scalar.activation(
                        ez.rearrange("p a b c -> p (a b c)"),
                        zt.rearrange("p a b c -> p (a b c)"), Exp)
                    live[b] = ez

                def emit_tail(b):
                    ez = live.pop(b)
                    da = da_pool.tile([128, 2, 256], f32, name="da", tag="da")
                    for nh in range(NH):
                        for h in range(2):
                            nc.tensor.matmul(
                                da[nh * 64:(nh + 1) * 64, 0, :],
                                lhsT=ones_bf[:, :],
                                rhs=ez[:, nh, h, :],
                                start=(h == 0), stop=(h == 1),
                                tile_position=(0, nh * 64))
                    for nh in range(NH):
                        bn = b * NH + nh
                        for kk in range(2):
                            nc.tensor.matmul(
                                da[nh * 64:(nh + 1) * 64, 1, :],
                                lhsT=v_all[:, bn, kk, :],
                                rhs=ez[:, nh, kk, :],
                                start=(kk == 0), stop=(kk == 1),
                                tile_position=(0, nh * 64))
                    rep = rp_pool.tile([128, 256], f32, name="rep", tag="rep")
                    nc.vector.reciprocal_approx_fast(rep[:, :], da[:, 0, :])
                    nc.vector.tensor_mul(
                        ut[:, b, :, :].rearrange("p a b -> p (a b)"),
                        da[:, 1, :], rep[:, :])
                    if b % 4 == 3:
                        bg = b // 4
                        rp = r_pool.tile([128, 512], f32, name="rp", tag="rp")
                        for nh in range(NH):
                            for jh in range(2):
                                nc.tensor.matmul(
                                    rp[nh * 64:(nh + 1) * 64, :],
                                    lhsT=wres_sb[nh * 64:(nh + 1) * 64, jh, :],
                                    rhs=ut[nh * 64:(nh + 1) * 64,
                                           bg * 4:(bg + 1) * 4, jh, :],
                                    start=(jh == 0), stop=(jh == 1),
                                    tile_position=(nh * 64, nh * 64))
                        fo = f_pool.tile([128, 512], f16, name="fo", tag="fo")
                        nc.scalar.activation(fo[:, :], rp[:, :], Relu,
                                             bias=bias_sb[:, :])
                        nc.sync.dma_start(out_d[:, bg, :], fo[:, :])

                emit_z(0)
                for b in range(BC):
                    if b + 1 < BC:
                        emit_z(b + 1)
                    emit_tail(b)
    nc.compile()
    return nc


def _get_nc():
    global _NC_CACHE
    if _NC_CACHE is None:
        _NC_CACHE = build_bass()
    return _NC_CACHE


def _prep_weight(W):
    # (CD, ND) -> (128, TTILES, KTILES*128): [p, t, kt*128+j] = W[kt*128+p, t*128+j]
    return np.ascontiguousarray(
        W.astype(np.float16).reshape(KTILES, 128, TTILES, 128)
        .transpose(1, 2, 0, 3).reshape(128, TTILES, KTILES * 128))


def make_in_maps(Hs, Wq, Wk, Wv, Wres_w, Wres_b):
    wq16 = _prep_weight(Wq)
    wk16 = _prep_weight(Wk)
    wv16 = Wv.astype(np.float16)
    wres16 = Wres_w.astype(np.float16)
    bias = Wres_b.astype(np.float32).reshape(E, 1)
    hs16 = Hs.astype(np.float16)
    maps = []
    for c in range(NCORES):
        sh = hs16[c * BC:(c + 1) * BC]                      # (BC, S, CD)
        hs2d = sh.reshape(ROWS, CD)
        hst = np.ascontiguousarray(
            hs2d.reshape(ROWS, KTILES, 128).transpose(2, 1, 0))
        # v rows in sigma' order (f*4+sp):
        # hsv[pi*64+e, q, f*4+sp] = Hs[b, nh*4+sp, f, e]; bn = 2q+pi = b*NH+nh
        arr = sh.reshape(NB, 4, F, E).transpose(0, 2, 1, 3).reshape(NB, 128, E)
        hsv = np.ascontiguousarray(
            arr.reshape(NB // 2, 2, 128, E).transpose(1, 3, 0, 2)
            .reshape(128, NB // 2, 128))
        maps.append({
            "hst": hst, "hsv": hsv,
            "wq": wq16, "wk": wk16, "wv": wv16, "wres": wres16, "bias": bias,
        })
    return maps


def _unpack_out(o):
    # o: (128, BC//4, 512) = ((nh, e), bg, (b4, f, sp)) -> (BC, S, F*E)
    o = np.asarray(o, dtype=np.float32).reshape(NH, E, BC // 4, 4, F, 4)
    return np.ascontiguousarray(
        o.transpose(2, 3, 0, 5, 4, 1)).reshape(BC, S, F * E)


def kernel(Hs, Wq, Wk, Wv, Wres_w, Wres_b):
    from concourse.bass_utils import run_bass_kernel_spmd
    nc = _get_nc()
    in_maps = make_in_maps(Hs, Wq, Wk, Wv, Wres_w, Wres_b)
    res = run_bass_kernel_spmd(nc, in_maps, list(range(NCORES)))
    out = np.concatenate(
        [_unpack_out(np.asarray(res.results[c]["out"]))
         for c in range(NCORES)], axis=0)
    return out.astype(np.float32)


if __name__ == "__main__":
    nc = build_bass()
    print("built OK; instructions:",
          sum(len(bb.instructions) for fn in nc.m.functions
              for bb in fn.blocks))


# revision 15
# speedup vs baseline: 1.1306x; 1.0293x over previous
"""Trainium2 Bass kernel for nn_MultiHeadSelfAttention_88725434400988.

R2 bisect: baseline projection/v-proj/startup + NEW attention section
(pipelined emission, shared dpr/av bank, 128-part resid, fp16 out).
"""
import numpy as np

B, S, F, E, A, NH = 256, 8, 32, 64, 64, 2
NCORES = 8
BC = B // NCORES            # 32 batches per core
ROWS = BC * S               # 256 projection rows
CD = F * E                  # 2048 contraction dim
ND = A * F * NH             # 4096 projection cols
KTILES = CD // 128          # 16
TTILES = ND // 128          # 32 column tiles per weight
NB = BC * NH                # 64 attention batches per core
WCHUNK = 2                  # weight tiles per DMA
GT = 2                      # projection tiles batched per psum/copy group

_NC_CACHE = None


def build_bass():
    import concourse.bacc as bacc
    import concourse.tile as tile
    from concourse import mybir

    f16 = mybir.dt.float16
    bf16 = mybir.dt.bfloat16
    f32 = mybir.dt.float32
    Exp = mybir.ActivationFunctionType.Exp
    Relu = mybir.ActivationFunctionType.Relu

    nc = bacc.Bacc("TRN2", target_bir_lowering=False, debug=False)

    # host-prepped layouts (see make_in_maps)
    hst_d = nc.dram_tensor("hst", [128, KTILES, ROWS], f16, kind="ExternalInput")
    hsv_d = nc.dram_tensor("hsv", [128, NB // 2, 128], f16, kind="ExternalInput")
    wq_d = nc.dram_tensor("wq", [128, TTILES, KTILES * 128], f16,
                          kind="ExternalInput")
    wk_d = nc.dram_tensor("wk", [128, TTILES, KTILES * 128], f16,
                          kind="ExternalInput")
    wv_d = nc.dram_tensor("wv", [E, 2 * A], f16, kind="ExternalInput")
    wres_d = nc.dram_tensor("wres", [2 * A, E], f16, kind="ExternalInput")
    bias_d = nc.dram_tensor("bias", [E, 1], f32, kind="ExternalInput")
    out_d = nc.dram_tensor("out", [128, BC // 4, 512], f16,
                           kind="ExternalOutput")

    with tile.TileContext(nc) as tc:
        from contextlib import ExitStack
        with ExitStack() as ctx:
            singles = ctx.enter_context(tc.tile_pool(name="singles", bufs=1))

            # ---- constants / persistent tiles ----
            ones_bf = singles.tile([128, A], bf16)
            nc.vector.memset(ones_bf, 1.0)

            hsT = singles.tile([128, KTILES, ROWS], f16)
            for c in range(4):
                nc.gpsimd.dma_start(hsT[:, c * 4:(c + 1) * 4, :],
                                    hst_d[:, c * 4:(c + 1) * 4, :])
            hsv = singles.tile([128, NB // 2, 128], f16)

            wv_sb = singles.tile([128, 2 * A], f16)
            nc.gpsimd.dma_start(wv_sb[0:64, :], wv_d[:])
            nc.gpsimd.dma_start(wv_sb[64:128, :], wv_d[:])

            wres_sb = singles.tile([128, 2, E], f16)
            for half in range(2):
                for jh in range(2):
                    nc.gpsimd.dma_start(
                        wres_sb[half * 64:(half + 1) * 64, jh, :],
                        wres_d[jh * 64:(jh + 1) * 64, :])

            bias_sb = singles.tile([128, 1], f32)
            nc.gpsimd.dma_start(bias_sb[0:64, :], bias_d[:])
            nc.gpsimd.dma_start(bias_sb[64:128, :], bias_d[:])

            qt = singles.tile([64, 2, BC, NH, 128], f16)
            kt_ = singles.tile([64, 2, BC, NH, 128], f16)
            v_all = singles.tile([128, NB, 2, A], bf16)
            ut = singles.tile([128, BC, 2, 128], f16)  # (nh,a) x (b, jh, f*4+sp)

            # ---- Q/K projection + batched gathers; the v projection
            # block sits between the two weight streams so its psum-evac
            # casts drain under the Wk stream and its matmuls bridge the
            # Wq->Wk DMA boundary. ----
            with tc.tile_pool(name="wtile", bufs=4) as w_pool, \
                 tc.tile_pool(name="stage", bufs=2) as st_pool, \
                 tc.tile_pool(name="pp", bufs=3, space="PSUM") as pp_pool, \
                 tc.tile_pool(name="vps", bufs=2, space="PSUM") as vps_pool:

                def v_proj_block():
                    # Per 4-bn group: the two PE row-group streams go to
                    # separate psum tiles (alternating row-group matmuls
                    # into ONE tile abort the run), two bpairs per tile,
                    # casts split across Vector/Scalar to keep PE dense.
                    for q in range(NB // 4):
                        vp = [vps_pool.tile([128, 2, 2, 64], f32,
                                            name=f"vp{i}", tag=f"vp{i}")
                              for i in range(2)]
                        for j in range(4):
                            pi = j % 2
                            nc.tensor.matmul(
                                vp[pi][:, j // 2, :, :],
                                lhsT=hsv[pi * 64:(pi + 1) * 64,
                                         2 * q + j // 2, :],
                                rhs=wv_sb[pi * 64:(pi + 1) * 64, :],
                                start=True, stop=True,
                                tile_position=(pi * 64, 0))
                        for pi in range(2):
                            cp = (nc.vector.tensor_copy if pi == 0
                                  else nc.scalar.copy)
                            cp(v_all[:, 4 * q + pi:4 * q + 4:2, :, :],
                               vp[pi][:, :, :, :])

                for widx, (w_d, dest) in enumerate(((wq_d, qt), (wk_d, kt_))):
                    stage = st_pool.tile([128, BC, NH, 128], f16,
                                         name="stage", tag="stage")
                    for tg in range(TTILES // WCHUNK):
                        dma_eng = nc.sync if tg % 2 == 0 else nc.gpsimd
                        wt = w_pool.tile([128, WCHUNK, KTILES, 128], f16,
                                         name="wt", tag="wt")
                        wsrc = (w_d[:, tg * WCHUNK:(tg + 1) * WCHUNK, :]
                                .rearrange("p t (kt c) -> p t kt c", c=128))
                        if widx == 0 and tg == 0:
                            # split the gating first chunk so MMs start
                            # after half the data
                            for h2 in range(WCHUNK):
                                dma_eng.dma_start(
                                    wt[:, h2:h2 + 1, :, :],
                                    wsrc[:, h2:h2 + 1, :, :])
                        else:
                            dma_eng.dma_start(wt[:, :, :, :], wsrc)
                        for gi in range(WCHUNK // GT):
                            pp = pp_pool.tile([128, GT, ROWS], f32)
                            for ti in range(GT):
                                for kt in range(KTILES):
                                    nc.tensor.matmul(
                                        pp[:, ti, :],
                                        lhsT=wt[:, gi * GT + ti, kt, :],
                                        rhs=hsT[:, kt, :],
                                        start=(kt == 0),
                                        stop=(kt == KTILES - 1))
                            # psum free (ti, b, nh, sp) -> iterate (bn, ti, sp)
                            src = pp.rearrange(
                                "p ti (b n sp) -> p (b n) ti sp", n=NH, sp=4)
                            t0 = tg * WCHUNK + gi * GT
                            dv = dest[:, 0, :, :, :].rearrange(
                                "p b n (f sp) -> p (b n) f sp", sp=4)
                            sv = stage[:, :, :, :].rearrange(
                                "p b n (f sp) -> p (b n) f sp", sp=4)
                            nc.vector.tensor_copy(
                                dv[:, :, t0:t0 + GT, :], src[0:64])
                            nc.vector.tensor_copy(
                                sv[64:128, :, t0:t0 + GT, :], src[64:128])
                            if widx == 0 and tg == 6:
                                # hsv is not needed until the v block at the
                                # end of the Wq stream; issuing it here keeps
                                # it out of the startup-critical ring window
                                nc.gpsimd.dma_start(hsv[:, :, :], hsv_d[:])
                    # partition shift 64..127 -> 0..63 via SBUF->SBUF DMA,
                    # chunked; the Wq shift overlaps the Wk stream.
                    nchunks = 2 if widx == 0 else 8
                    bchunk = BC // nchunks
                    for c in range(nchunks):
                        eng = nc.gpsimd if widx == 0 else nc.sync
                        eng.dma_start(
                            dest[:, 1, c * bchunk:(c + 1) * bchunk, :, :],
                            stage[64:128, c * bchunk:(c + 1) * bchunk, :, :])
                    if widx == 0:
                        v_proj_block()

            # ---- PE warm-up burst ----
            with tc.tile_pool(name="warm", bufs=1, space="PSUM") as wm_pool:
                wm = wm_pool.tile([A, ROWS], f32)
                for wi in range(24):
                    nc.tensor.matmul(
                        wm[:, :], lhsT=ones_bf[:, :], rhs=hsT[:, 0, :],
                        start=(wi == 0), stop=(wi == 23))

            # ---- attention (transpose-free, Z^T layout), software
            # pipelined: z/exp for b+1 are emitted before denom/AV of b ----
            with tc.tile_pool(name="zps", bufs=2, space="PSUM") as z_pool, \
                 tc.tile_pool(name="dav", bufs=2, space="PSUM") as da_pool, \
                 tc.tile_pool(name="rps", bufs=2, space="PSUM") as r_pool, \
                 tc.tile_pool(name="expz", bufs=3) as e_pool, \
                 tc.tile_pool(name="reps", bufs=3) as rp_pool, \
                 tc.tile_pool(name="fo", bufs=2) as f_pool:

                live = {}

                def emit_z(b):
                    zt = z_pool.tile([128, NH, 2, 256], f32, name="zt",
                                     tag="zt")
                    for nh in range(NH):
                        for h in range(2):
                            nc.tensor.matmul(
                                zt[:, nh, h, :],
                                lhsT=kt_[:, h, b, nh, :],
                                rhs=qt[:, :, b, nh, :],
                                start=True, stop=True)
                    ez = e_pool.tile([128, NH, 2, 256], bf16, name="ez",
                                     tag="ez")
                    nc.scalar.activation(
                        ez.rearrange("p a b c -> p (a b c)"),
                        zt.rearrange("p a b c -> p (a b c)"), Exp)
                    live[b] = ez

                def emit_tail(b):
                    ez = live.pop(b)
                    da = da_pool.tile([128, 2, 256], f32, name="da", tag="da")
                    for nh in range(NH):
                        for h in range(2):
                            nc.tensor.matmul(
                                da[nh * 64:(nh + 1) * 64, 0, :],
                                lhsT=ones_bf[:, :],
                                rhs=ez[:, nh, h, :],
                                start=(h == 0), stop=(h == 1),
                                tile_position=(0, nh * 64))
                    for nh in range(NH):
                        bn = b * NH + nh
                        for kk in range(2):
                            nc.tensor.matmul(
                                da[nh * 64:(nh + 1) * 64, 1, :],
                                lhsT=v_all[:, bn, kk, :],
                                rhs=ez[:, nh, kk, :],
                                start=(kk == 0), stop=(kk == 1),
                                tile_position=(0, nh * 64))
                    rep = rp_pool.tile([128, 256], f32, name="rep", tag="rep")
                    nc.vector.reciprocal_approx_fast(rep[:, :], da[:, 0, :])
                    nc.vector.tensor_mul(
                        ut[:, b, :, :].rearrange("p a b -> p (a b)"),
                        da[:, 1, :], rep[:, :])
                    if b % 4 == 3:
                        bg = b // 4
                        rp = r_pool.tile([128, 512], f32, name="rp", tag="rp")
                        for nh in range(NH):
                            for jh in range(2):
                                nc.tensor.matmul(
                                    rp[nh * 64:(nh + 1) * 64, :],
                                    lhsT=wres_sb[nh * 64:(nh + 1) * 64, jh, :],
                                    rhs=ut[nh * 64:(nh + 1) * 64,
                                           bg * 4:(bg + 1) * 4, jh, :],
                                    start=(jh == 0), stop=(jh == 1),
                                    tile_position=(nh * 64, nh * 64))
                        fo = f_pool.tile([128, 512], f16, name="fo", tag="fo")
                        nc.scalar.activation(fo[:, :], rp[:, :], Relu,
                                             bias=bias_sb[:, :])
                        nc.sync.dma_start(out_d[:, bg, :], fo[:, :])

                emit_z(0)
                for b in range(BC):
                    if b + 1 < BC:
                        emit_z(b + 1)
                    emit_tail(b)
    nc.compile()
    return nc


def _get_nc():
    global _NC_CACHE
    if _NC_CACHE is None:
        _NC_CACHE = build_bass()
    return _NC_CACHE


def _prep_weight(W):
    # (CD, ND) -> (128, TTILES, KTILES*128): [p, t, kt*128+j] = W[kt*128+p, t*128+j]
    return np.ascontiguousarray(
        W.astype(np.float16).reshape(KTILES, 128, TTILES, 128)
        .transpose(1, 2, 0, 3).reshape(128, TTILES, KTILES * 128))


def make_in_maps(Hs, Wq, Wk, Wv, Wres_w, Wres_b):
    wq16 = _prep_weight(Wq)
    wk16 = _prep_weight(Wk)
    wv16 = Wv.astype(np.float16)
    wres16 = Wres_w.astype(np.float16)
    bias = Wres_b.astype(np.float32).reshape(E, 1)
    hs16 = Hs.astype(np.float16)
    maps = []
    for c in range(NCORES):
        sh = hs16[c * BC:(c + 1) * BC]                      # (BC, S, CD)
        hs2d = sh.reshape(ROWS, CD)
        hst = np.ascontiguousarray(
            hs2d.reshape(ROWS, KTILES, 128).transpose(2, 1, 0))
        # v rows in sigma' order (f*4+sp):
        # hsv[pi*64+e, q, f*4+sp] = Hs[b, nh*4+sp, f, e]; bn = 2q+pi = b*NH+nh
        arr = sh.reshape(NB, 4, F, E).transpose(0, 2, 1, 3).reshape(NB, 128, E)
        hsv = np.ascontiguousarray(
            arr.reshape(NB // 2, 2, 128, E).transpose(1, 3, 0, 2)
            .reshape(128, NB // 2, 128))
        maps.append({
            "hst": hst, "hsv": hsv,
            "wq": wq16, "wk": wk16, "wv": wv16, "wres": wres16, "bias": bias,
        })
    return maps


def _unpack_out(o):
    # o: (128, BC//4, 512) = ((nh, e), bg, (b4, f, sp)) -> (BC, S, F*E)
    o = np.asarray(o, dtype=np.float32).reshape(NH, E, BC // 4, 4, F, 4)
    return np.ascontiguousarray(
        o.transpose(2, 3, 0, 5, 4, 1)).reshape(BC, S, F * E)


def kernel(Hs, Wq, Wk, Wv, Wres_w, Wres_b):
    from concourse.bass_utils import run_bass_kernel_spmd
    nc = _get_nc()
    in_maps = make_in_maps(Hs, Wq, Wk, Wv, Wres_w, Wres_b)
    res = run_bass_kernel_spmd(nc, in_maps, list(range(NCORES)))
    out = np.concatenate(
        [_unpack_out(np.asarray(res.results[c]["out"]))
         for c in range(NCORES)], axis=0)
    return out.astype(np.float32)


if __name__ == "__main__":
    nc = build_bass()
    print("built OK; instructions:",
          sum(len(bb.instructions) for fn in nc.m.functions
              for bb in fn.blocks))
